# revision 73
# baseline (speedup 1.0000x reference)
"""Multi-head self-attention Trainium2 kernel (8 NeuronCores, SPMD).

Problem: B=4, S=2048, H=1024, 16 heads (dh=64), fp32 I/O.
Sharding: core c = b*2 + g handles batch b and head-group g (8 heads).
Each core computes a partial output Y_g = softmax(QK^T/sqrt(d), mask) V W_o[g]
for its 8 heads; the host sums the two partials per batch and adds b_o.

v2 design notes (from NTFF profile analysis of v1):
- ACT (exp over all 33.5M score elems/core) is a hard ~300us floor at
  1 elem/lane/cycle @1.2GHz; the goal is to keep every other engine hidden
  underneath back-to-back exps.
- The two K=64 score matmuls of a head pair auto-row-tile (base partitions
  0/64) and run concurrently in the PE array.
- Phases are tp-major (pi = tp*4 + qc) so only head-pair 0's Q/K
  projections gate loop start; m2/m3 are re-projected in-loop from
  re-DMA'd x tiles (whole-row [128,S] DMAs only - chunked column DMAs
  have 1KB segments and run ~4x slower).
- Tile's scheduler reorders by priority (emission order) + cost model;
  buffers are sized so it can pack V projection and Q/K m1 fillers under
  the ACT-bound steady state.
- PSUM: ps_a = 2 x [128,2,512] score tiles (4 banks, double buffer);
  ps_b = 4 x [128,512] (PV accumulators + projection/Y transients).
"""

import os
import sys
from collections import deque
from contextlib import ExitStack

sys.path.insert(0, "/opt/trn_rl_repo")

import numpy as np
import ml_dtypes

import concourse.bass as bass
import concourse.tile as tile
from concourse import bacc
from concourse import mybir
from concourse.bass_utils import run_bass_kernel_spmd

BF16 = ml_dtypes.bfloat16

# Geometry (hardcoded for this problem)
S = 2048          # sequence length
HIN = 1024        # model hidden
F = 512           # per-core features = 8 heads * 64
NH = 8            # heads per core
DH = 64           # head dim
HOUT = 1024       # output hidden
QC = 512          # q chunk
NQC = S // QC     # 4
NKT = 16          # key tiles of 128
NJIN = HIN // 128  # 8 contraction tiles for projections
NPF = F // 128     # 4 feature ptiles (2 heads each)
NPH = NPF * NQC    # 16 phases
NSTEP = NPH * NKT  # 256

f32 = mybir.dt.float32
bf16 = mybir.dt.bfloat16
EXPF = mybir.ActivationFunctionType.Exp


def _attention_body(ctx, tc, io):
    nc = tc.nc
    xdrams, maskT, xvP, ws, bs, y = io

    consts = ctx.enter_context(tc.tile_pool(name="consts", bufs=1))
    wpool = ctx.enter_context(tc.tile_pool(name="wpool", bufs=1))
    xpool = ctx.enter_context(tc.tile_pool(name="xpool", bufs=8))
    xvsub = ctx.enter_context(tc.tile_pool(name="xvsub", bufs=2))
    qkvp = ctx.enter_context(tc.tile_pool(name="qkvp", bufs=1))
    mpool = ctx.enter_context(tc.tile_pool(name="mpool", bufs=3))
    ppool = ctx.enter_context(tc.tile_pool(name="ppool", bufs=10))
    outp = ctx.enter_context(tc.tile_pool(name="outp", bufs=1))
    ypool = ctx.enter_context(tc.tile_pool(name="ypool", bufs=2))
    norm_s = ctx.enter_context(tc.tile_pool(name="norm_s", bufs=1))
    norm_r = ctx.enter_context(tc.tile_pool(name="norm_r", bufs=1))
    ps_a = ctx.enter_context(tc.tile_pool(name="ps_a", bufs=2, space="PSUM"))
    ps_b = ctx.enter_context(tc.tile_pool(name="ps_b", bufs=4, space="PSUM"))

    # ---- constants + ACT exp-table warmup (table load ~2.7us hides in DMA)
    warm_in = consts.tile([1, 32], f32, tag="warm_in", name="warm_in")
    warm_out = consts.tile([1, 32], bf16, tag="warm_out", name="warm_out")
    nc.vector.memset(warm_in, 0.0)
    nc.scalar.activation(out=warm_out, in_=warm_in, func=EXPF, scale=0.125)

    ones_col = consts.tile([1, 128], bf16, tag="ones_col", name="ones_col")
    nc.vector.memset(ones_col, 1.0)

    wq_sb = wpool.tile([128, NJIN, F], bf16, tag="wq", name="wq")
    wk_sb = wpool.tile([128, NJIN, F], bf16, tag="wk", name="wk")
    wv_sb = wpool.tile([128, NJIN, F], bf16, tag="wv", name="wv")
    wo_sb = wpool.tile([128, NPF, HOUT], bf16, tag="wo", name="wo")
    bqc_sb = consts.tile([128, NPF], f32, tag="bqc", name="bqc")
    bkc_sb = consts.tile([128, NPF], f32, tag="bkc", name="bkc")
    bv_sb = consts.tile([1, F], bf16, tag="bv", name="bv")

    # ---- persistent SBUF tensors
    qt_sb = [qkvp.tile([128, S], bf16, tag=f"qt{m}", name=f"qt{m}") for m in range(NPF)]
    kt_sb = [qkvp.tile([128, S], bf16, tag=f"kt{m}", name=f"kt{m}") for m in range(NPF)]
    # flat V tiles: head h at columns [h*65, h*65+64]; padded to 583 so a
    # 128-column stationary window (FWL-eligible) can start at any head
    v_sb = [qkvp.tile([128, NH * (DH + 1) + 63], bf16, tag=f"v{t}", name=f"v{t}")
            for t in range(NKT)]
    out_sbs = [[outp.tile([128, QC], bf16, tag=f"o{qc}_{m}", name=f"o{qc}_{m}")
                for m in range(NPF)] for qc in range(NQC)]

    # mask tiles: per phase, two half-chunks [128, 8, QC] (keys x kt x q)
    m_tiles = {}

    def load_mask_half(pi, h):
        # host pre-permuted mask: row qc*128+p holds [kt, q] contiguous
        qc = pi % NQC
        mt = mpool.tile([128, NKT // 2, QC], bf16, tag="mask", name="mask")
        nc.sync.dma_start(
            out=mt,
            in_=maskT[qc * 128:(qc + 1) * 128, h * 8:(h + 1) * 8, :],
        )
        m_tiles.setdefault(pi, {})[h] = mt

    def load_x(xname):
        tiles = []
        for j in range(NJIN):
            xt = xpool.tile([128, S], bf16, tag="x", name="x")
            nc.sync.dma_start(out=xt, in_=xdrams[xname][j * 128:(j + 1) * 128, :])
            tiles.append(xt)
        return tiles

    # ---- DMA emission order = queue order: xq, masks(0), xk, xv, wo, mask(1)
    nc.sync.dma_start(out=wq_sb, in_=ws["wq"][:, :, :])
    nc.sync.dma_start(out=bqc_sb, in_=bs["bqc"][:, :])
    nc.sync.dma_start(out=wk_sb, in_=ws["wk"][:, :, :])
    nc.sync.dma_start(out=bkc_sb, in_=bs["bkc"][:, :])
    nc.sync.dma_start(out=wv_sb, in_=ws["wv"][:, :, :])
    nc.sync.dma_start(out=bv_sb, in_=bs["bv"][:, :])
    xq_tiles = load_x("xqT")
    xk_tiles = load_x("xkT")
    xv_chunks = []
    for g in range(NH):
        xt = xvsub.tile([128, NJIN, 256], bf16, tag="xv", name="xv")
        nc.sync.dma_start(out=xt, in_=xvP[g * 128:(g + 1) * 128, :, :])
        xv_chunks.append(xt)
    load_mask_half(0, 0)
    load_mask_half(0, 1)
    nc.sync.dma_start(out=wo_sb, in_=ws["wo"][:, :, :])
    load_mask_half(1, 0)

    for t in range(NKT):
        nc.vector.memset(v_sb[t], 1.0)  # ones column for the PV rowsum trick

    # ---- Q/K projection pass for m=0 and m=1, j-streamed over arriving x
    def qk_pass01(x_tiles, w_sb, bcp, t_out):
        pas = [ps_a.tile([128, 2, QC], f32, tag="sc", name="sc") for _ in range(2)]
        pbs = [ps_b.tile([128, QC], f32, tag="pb", name="pb") for _ in range(NQC)]
        for j in range(NJIN):
            for h in range(2):
                for i in range(2):
                    nc.tensor.matmul(
                        pas[h][:, i, :],
                        lhsT=w_sb[:, j, 0:128],
                        rhs=x_tiles[j][:, (2 * h + i) * QC:(2 * h + i + 1) * QC],
                        start=(j == 0),
                        stop=(j == NJIN - 1),
                    )
            for n in range(NQC):
                nc.tensor.matmul(
                    pbs[n],
                    lhsT=w_sb[:, j, 128:256],
                    rhs=x_tiles[j][:, n * QC:(n + 1) * QC],
                    start=(j == 0),
                    stop=(j == NJIN - 1),
                )
        for h in range(2):
            nc.scalar.add(
                out=t_out[0][:, 2 * h * QC:(2 * h + 2) * QC],
                in_=pas[h].rearrange("p a b -> p (a b)"),
                add=bcp[:, 0:1])
        for n in range(NQC):
            nc.vector.tensor_scalar_add(
                out=t_out[1][:, n * QC:(n + 1) * QC], in0=pbs[n],
                scalar1=bcp[:, 1:2])

    qk_pass01(xq_tiles, wq_sb, bqc_sb, qt_sb)
    qk_pass01(xk_tiles, wk_sb, bkc_sb, kt_sb)

    # ---- V projection in 2-tile groups, streamed against the xvP chunks
    for g in range(NH):
        xt = xv_chunks[g]
        pss = [ps_b.tile([128, QC], f32, tag="pb", name="pb") for _ in range(2)]
        for j in range(NJIN):
            for i in range(2):
                nc.tensor.matmul(
                    pss[i],
                    lhsT=xt[:, j, i * 128:(i + 1) * 128],
                    rhs=wv_sb[:, j, :],
                    start=(j == 0),
                    stop=False,
                )
        for i in range(2):
            nc.tensor.matmul(pss[i], lhsT=ones_col, rhs=bv_sb,
                             start=False, stop=True)
        for i in range(2):
            vt = v_sb[2 * g + i]
            vview = bass.AP(tensor=vt.tensor, offset=vt.offset,
                            ap=[vt.ap[0], [DH + 1, NH], [1, DH]])
            nc.vector.tensor_scalar_mul(
                out=vview,
                in0=pss[i].rearrange("p (h d) -> p h d", h=NH),
                scalar1=1.0)

    # ---- in-loop fillers: Q/K m2,m3 from re-DMA'd x; then Y as it queues
    fillers = deque()
    fstate = {}

    def redma(xname):
        def run():
            fstate[xname] = load_x(xname)
        return run

    def qk_group(xname, w_sb, bcp, t_out, m, n):
        def run():
            x_tiles = fstate[xname]
            ps = ps_b.tile([128, QC], f32, tag="pb", name="pb")
            for j in range(NJIN):
                nc.tensor.matmul(
                    ps,
                    lhsT=w_sb[:, j, m * 128:(m + 1) * 128],
                    rhs=x_tiles[j][:, n * QC:(n + 1) * QC],
                    start=(j == 0),
                    stop=(j == NJIN - 1),
                )
            nc.vector.tensor_scalar_add(
                out=t_out[m][:, n * QC:(n + 1) * QC], in0=ps,
                scalar1=bcp[:, m:m + 1])
        return run

    fillers.append(redma("xqT"))
    for m in (2, 3):
        for n in range(NQC):
            fillers.append(qk_group("xqT", wq_sb, bqc_sb, qt_sb, m, n))
    fillers.append(redma("xkT"))
    for m in (2, 3):
        for n in range(NQC):
            fillers.append(qk_group("xkT", wk_sb, bkc_sb, kt_sb, m, n))
    y_fill = deque()

    # ---- main software-pipelined stream (phases tp-major: pi = tp*NQC + qc)
    pv_pss = {}       # pi -> [2 psum accumulators]
    p2s = {}          # step -> p2 tile
    norm_b_queue = []
    ysbs = {}

    def y_half(qc, qt, nch):
        def run():
            if nch == 0:
                ysbs[(qc, qt)] = ypool.tile([128, 2, QC], f32, tag="y", name="y")
            ysb = ysbs[(qc, qt)]
            ps = ps_b.tile([128, QC], f32, tag="pb", name="pb")
            for j in range(NPF):
                nc.tensor.matmul(
                    ps,
                    lhsT=out_sbs[qc][j][:, qt * 128:(qt + 1) * 128],
                    rhs=wo_sb[:, j, nch * QC:(nch + 1) * QC],
                    start=(j == 0),
                    stop=(j == NPF - 1),
                )
            nc.vector.tensor_scalar_mul(out=ysb[:, nch, :], in0=ps, scalar1=1.0)
            if nch == 1:
                del ysbs[(qc, qt)]
                r0 = qc * QC + qt * 128
                nc.sync.dma_start(out=y[r0:r0 + 128, :], in_=ysb)
        return run

    def lag(x):
        return 8 if x < 16 else max(4, 24 - x)

    def emit_sk(s):
        pi, kt = divmod(s, NKT)
        tp, qc = pi // NQC, pi % NQC
        if kt == 4 and pi + 1 < NPH and 0 not in m_tiles.get(pi + 1, {}):
            load_mask_half(pi + 1, 0)
        if kt == 10 and pi + 1 < NPH:
            load_mask_half(pi + 1, 1)
        qsl = slice(qc * QC, (qc + 1) * QC)
        sc = ps_a.tile([128, 2, QC], f32, tag="sc", name="sc")
        for sub in range(2):
            rsl = slice(sub * 64, (sub + 1) * 64)
            nc.tensor.matmul(
                sc[:, sub, :],
                lhsT=kt_sb[tp][rsl, kt * 128:(kt + 1) * 128],
                rhs=qt_sb[tp][rsl, qsl],
                start=True,
                stop=True,
            )
        p2 = ppool.tile([128, 2, QC], bf16, tag="p", name="p")
        nc.scalar.activation(out=p2, in_=sc, func=EXPF, scale=0.125)
        # one masked multiply for both heads: mask broadcast over the head dim
        mh = m_tiles[pi][kt // 8][:, kt % 8, :]
        mask_bc = bass.AP(tensor=mh.tensor, offset=mh.offset,
                          ap=[mh.ap[0], [0, 2], mh.ap[1]])
        nc.vector.tensor_tensor(out=p2, in0=p2, in1=mask_bc,
                                op=mybir.AluOpType.mult)
        p2s[s] = p2

    def emit_pv(s):
        pi, kt = divmod(s, NKT)
        tp, qc = pi // NQC, pi % NQC
        if kt == 0:
            pv_pss[pi] = [ps_b.tile([128, QC], f32, tag="pb", name="pb")
                          for _ in range(2)]
        p2 = p2s.pop(s)
        for sub in range(2):
            h0 = (2 * tp + sub) * (DH + 1)
            nc.tensor.matmul(
                pv_pss[pi][sub][0:128, :],
                lhsT=v_sb[kt][:, h0:h0 + 128],
                rhs=p2[:, sub, :],
                start=(kt == 0),
                stop=(kt == NKT - 1),
            )
        if kt == NKT - 1:
            emit_norm_a(pi, s)

    def norm_rec(pi, sub, recbs):
        # rowsums live at psum partition 64 of each PV accumulator
        rsum = norm_s.tile([1, QC], f32, tag=f"rsum{sub}", name="rsum")
        nc.vector.tensor_copy(out=rsum, in_=pv_pss[pi][sub][DH:DH + 1, :])
        rec = norm_s.tile([1, QC], f32, tag=f"rec{sub}", name="rec")
        scr = norm_s.tile([1, QC], f32, tag="scr", name="scr")
        nc.vector.reciprocal_approx_accurate(out=rec, in_=rsum, scratch=scr)
        recb = norm_r.tile([64, QC], f32, tag=f"recb{sub}", name="recb")
        rec_bc = bass.AP(tensor=rec.tensor, offset=rec.offset,
                         ap=[rec.ap[0], [0, 64], rec.ap[1]])
        # idle GpSimd SWDGE queue: skips the busy sync-DMA queue backlog
        nc.gpsimd.dma_start(out=recb, in_=rec_bc)
        recbs.append(recb)

    def norm_mult(pi, sub, recbs):
        tp, qc = pi // NQC, pi % NQC
        rsl = slice(sub * 64, (sub + 1) * 64)
        nc.vector.tensor_tensor(
            out=out_sbs[qc][tp][rsl, :],
            in0=pv_pss[pi][sub][0:DH, :],
            in1=recbs[sub],
            op=mybir.AluOpType.mult,
        )
        if sub == 1:
            pv_pss.pop(pi)
            tpq, qcq = pi // NQC, pi % NQC
            if tpq == NPF - 1:
                for qt in range(NQC):
                    for nch in range(2):
                        y_fill.append(y_half(qcq, qt, nch))

    def emit_norm_a(pi, s):
        # spread the norm work one op-group per step to avoid a DVE burst
        recbs = []
        norm_b_queue.append((s + 1, lambda: norm_rec(pi, 0, recbs)))
        norm_b_queue.append((s + 2, lambda: norm_rec(pi, 1, recbs)))
        norm_b_queue.append((s + 6, lambda: norm_mult(pi, 0, recbs)))
        norm_b_queue.append((s + 7, lambda: norm_mult(pi, 1, recbs)))

    def emit_norm_b():
        _, fn = norm_b_queue.pop(0)
        fn()

    pv_next = 0
    for s in range(NSTEP + 96):
        if s < NSTEP:
            emit_sk(s)
        while norm_b_queue and norm_b_queue[0][0] <= s:
            emit_norm_b()
        while pv_next < NSTEP and pv_next <= s - lag(pv_next):
            emit_pv(pv_next)
            pv_next += 1
        while norm_b_queue and norm_b_queue[0][0] <= s:
            emit_norm_b()
        # fillers: keep clear of the PV-accumulator handover window (kt 4-8)
        kt = s % NKT
        if not (4 <= kt <= 8) or s >= NSTEP:
            if fillers and (s >= 28 and s % 3 == 0 or s >= NSTEP):
                fillers.popleft()()
            elif y_fill:
                y_fill.popleft()()
                if s >= NSTEP and y_fill:
                    y_fill.popleft()()
        if s >= NSTEP and pv_next >= NSTEP and not norm_b_queue and \
                not fillers and not y_fill:
            break
    while norm_b_queue:
        emit_norm_b()
    while y_fill:
        y_fill.popleft()()


_NC_CACHE = None


def _build_nc():
    global _NC_CACHE
    if _NC_CACHE is None:
        nc = bacc.Bacc("TRN2", target_bir_lowering=False, name="mhsa")
        xdrams = {
            n: nc.declare_dram_parameter(n, [HIN, S], bf16, isOutput=False)
            for n in ("xqT", "xkT")
        }
        maskT = nc.declare_dram_parameter(
            "maskT", [NQC * 128, NKT, QC], bf16, isOutput=False)
        xvP = nc.declare_dram_parameter(
            "xvP", [NH * 128, NJIN, 256], bf16, isOutput=False)
        ws = {
            "wq": nc.declare_dram_parameter("wq", [128, NJIN, F], bf16, isOutput=False),
            "wk": nc.declare_dram_parameter("wk", [128, NJIN, F], bf16, isOutput=False),
            "wv": nc.declare_dram_parameter("wv", [128, NJIN, F], bf16, isOutput=False),
            "wo": nc.declare_dram_parameter("wo", [128, NPF, HOUT], bf16, isOutput=False),
        }
        bs = {
            "bqc": nc.declare_dram_parameter("bqc", [128, NPF], f32, isOutput=False),
            "bkc": nc.declare_dram_parameter("bkc", [128, NPF], f32, isOutput=False),
            "bv": nc.declare_dram_parameter("bv", [1, F], bf16, isOutput=False),
        }
        y = nc.declare_dram_parameter("y", [S, HOUT], f32, isOutput=True)
        with tile.TileContext(nc) as tc:
            with ExitStack() as ctx:
                _attention_body(ctx, tc, (xdrams, maskT, xvP, ws, bs, y))
        nc.compile()
        _NC_CACHE = nc
    return _NC_CACHE


LAST_RESULTS = None


def kernel(queries, keys, values, attention_mask,
           W_q, b_q, W_k, b_k, W_v, b_v, W_o, b_o):
    global LAST_RESULTS
    nc = _build_nc()

    B = queries.shape[0]
    n_cores = 2 * B

    def prep_w(W, g):
        Wg = np.asarray(W[:, g * F:(g + 1) * F], np.float32).astype(BF16)
        return np.ascontiguousarray(Wg.reshape(NJIN, 128, F).transpose(1, 0, 2))

    def prep_wo(W, g):
        Wg = np.asarray(W[g * F:(g + 1) * F, :], np.float32).astype(BF16)
        return np.ascontiguousarray(Wg.reshape(NPF, 128, HOUT).transpose(1, 0, 2))

    def prep_bcol(b, g):
        bg = np.asarray(b[g * F:(g + 1) * F], np.float32)
        return np.ascontiguousarray(bg.reshape(NPF, 128).T)

    in_maps = []
    for b in range(B):
        xqT_ = np.ascontiguousarray(np.asarray(queries[b], np.float32).astype(BF16).T)
        xkT_ = np.ascontiguousarray(np.asarray(keys[b], np.float32).astype(BF16).T)
        # xvP[g*128+p, j, tok] = X_v^T[j*128+p, g*256+tok] (contiguous V chunks)
        xvP_ = np.ascontiguousarray(
            np.asarray(values[b], np.float32).astype(BF16).T
            .reshape(NJIN, 128, NH, 256).transpose(2, 1, 0, 3)
            .reshape(NH * 128, NJIN, 256))
        # permute mask so each (qc, key-tile-half) chunk is DMA-contiguous:
        # maskT_[qc*128+p, t, ql] = mask[qc*512+ql, t*128+p]
        maskT_ = np.ascontiguousarray(
            np.asarray(attention_mask[b]).astype(np.float32).astype(BF16)
            .reshape(NQC, QC, NKT, 128).transpose(0, 3, 2, 1)
            .reshape(NQC * 128, NKT, QC))
        for g in range(2):
            in_maps.append({
                "xqT": xqT_, "xkT": xkT_, "xvP": xvP_, "maskT": maskT_,
                "wq": prep_w(W_q, g), "wk": prep_w(W_k, g), "wv": prep_w(W_v, g),
                "wo": prep_wo(W_o, g),
                "bqc": prep_bcol(b_q, g),
                "bkc": prep_bcol(b_k, g),
                "bv": np.asarray(b_v[g * F:(g + 1) * F], np.float32).astype(BF16).reshape(1, F),
            })

    res = run_bass_kernel_spmd(
        nc, in_maps, list(range(n_cores)),
        trace=bool(os.environ.get("MHSA_TRACE")),
    )
    LAST_RESULTS = res

    out = np.empty((B, S, HOUT), np.float32)
    bo = np.asarray(b_o, np.float32)
    for b in range(B):
        out[b] = res.results[2 * b]["y"] + res.results[2 * b + 1]["y"] + bo
    return out


# revision 74
# speedup vs baseline: 1.0095x; 1.0095x over previous
"""Multi-head self-attention Trainium2 kernel (8 NeuronCores, SPMD).

Problem: B=4, S=2048, H=1024, 16 heads (dh=64), fp32 I/O.
Sharding: core c = b*2 + g handles batch b and head-group g (8 heads).
Each core computes a partial output Y_g = softmax(QK^T/sqrt(d), mask) V W_o[g]
for its 8 heads; the host sums the two partials per batch and adds b_o.

v2 design notes (from NTFF profile analysis of v1):
- ACT (exp over all 33.5M score elems/core) is a hard ~300us floor at
  1 elem/lane/cycle @1.2GHz; the goal is to keep every other engine hidden
  underneath back-to-back exps.
- The two K=64 score matmuls of a head pair auto-row-tile (base partitions
  0/64) and run concurrently in the PE array.
- Phases are tp-major (pi = tp*4 + qc) so only head-pair 0's Q/K
  projections gate loop start; m2/m3 are re-projected in-loop from
  re-DMA'd x tiles (whole-row [128,S] DMAs only - chunked column DMAs
  have 1KB segments and run ~4x slower).
- Tile's scheduler reorders by priority (emission order) + cost model;
  buffers are sized so it can pack V projection and Q/K m1 fillers under
  the ACT-bound steady state.
- PSUM: ps_a = 2 x [128,2,512] score tiles (4 banks, double buffer);
  ps_b = 4 x [128,512] (PV accumulators + projection/Y transients).
"""

import os
import sys
from collections import deque
from contextlib import ExitStack

sys.path.insert(0, "/opt/trn_rl_repo")

import numpy as np
import ml_dtypes

import concourse.bass as bass
import concourse.tile as tile
from concourse import bacc
from concourse import mybir
from concourse.bass_utils import run_bass_kernel_spmd

BF16 = ml_dtypes.bfloat16

# Geometry (hardcoded for this problem)
S = 2048          # sequence length
HIN = 1024        # model hidden
F = 512           # per-core features = 8 heads * 64
NH = 8            # heads per core
DH = 64           # head dim
HOUT = 1024       # output hidden
QC = 512          # q chunk
NQC = S // QC     # 4
NKT = 16          # key tiles of 128
NJIN = HIN // 128  # 8 contraction tiles for projections
NPF = F // 128     # 4 feature ptiles (2 heads each)
NPH = NPF * NQC    # 16 phases
NSTEP = NPH * NKT  # 256

f32 = mybir.dt.float32
bf16 = mybir.dt.bfloat16
EXPF = mybir.ActivationFunctionType.Exp


def _attention_body(ctx, tc, io):
    nc = tc.nc
    xdrams, maskT, xvP, ws, bs, y = io

    consts = ctx.enter_context(tc.tile_pool(name="consts", bufs=1))
    wpool = ctx.enter_context(tc.tile_pool(name="wpool", bufs=1))
    xpool = ctx.enter_context(tc.tile_pool(name="xpool", bufs=8))
    xvsub = ctx.enter_context(tc.tile_pool(name="xvsub", bufs=2))
    qkvp = ctx.enter_context(tc.tile_pool(name="qkvp", bufs=1))
    mpool = ctx.enter_context(tc.tile_pool(name="mpool", bufs=3))
    ppool = ctx.enter_context(tc.tile_pool(name="ppool", bufs=10))
    outp = ctx.enter_context(tc.tile_pool(name="outp", bufs=1))
    ypool = ctx.enter_context(tc.tile_pool(name="ypool", bufs=2))
    norm_s = ctx.enter_context(tc.tile_pool(name="norm_s", bufs=1))
    norm_r = ctx.enter_context(tc.tile_pool(name="norm_r", bufs=1))
    ps_a = ctx.enter_context(tc.tile_pool(name="ps_a", bufs=2, space="PSUM"))
    ps_b = ctx.enter_context(tc.tile_pool(name="ps_b", bufs=4, space="PSUM"))

    # ---- constants + ACT exp-table warmup (table load ~2.7us hides in DMA)
    warm_in = consts.tile([1, 32], f32, tag="warm_in", name="warm_in")
    warm_out = consts.tile([1, 32], bf16, tag="warm_out", name="warm_out")
    nc.vector.memset(warm_in, 0.0)
    nc.scalar.activation(out=warm_out, in_=warm_in, func=EXPF, scale=0.125)

    ones_col = consts.tile([1, 128], bf16, tag="ones_col", name="ones_col")
    nc.vector.memset(ones_col, 1.0)

    wq_sb = wpool.tile([128, NJIN, F], bf16, tag="wq", name="wq")
    wk_sb = wpool.tile([128, NJIN, F], bf16, tag="wk", name="wk")
    wv_sb = wpool.tile([128, NJIN, F], bf16, tag="wv", name="wv")
    wo_sb = wpool.tile([128, NPF, HOUT], bf16, tag="wo", name="wo")
    bqc_sb = consts.tile([128, NPF], f32, tag="bqc", name="bqc")
    bkc_sb = consts.tile([128, NPF], f32, tag="bkc", name="bkc")
    bv_sb = consts.tile([1, F], bf16, tag="bv", name="bv")

    # ---- persistent SBUF tensors
    qt_sb = [qkvp.tile([128, S], bf16, tag=f"qt{m}", name=f"qt{m}") for m in range(NPF)]
    kt_sb = [qkvp.tile([128, S], bf16, tag=f"kt{m}", name=f"kt{m}") for m in range(NPF)]
    # flat V tiles: head h at columns [h*65, h*65+64]; padded to 583 so a
    # 128-column stationary window (FWL-eligible) can start at any head
    v_sb = [qkvp.tile([128, NH * (DH + 1) + 63], bf16, tag=f"v{t}", name=f"v{t}")
            for t in range(NKT)]
    out_sbs = [[outp.tile([128, QC], bf16, tag=f"o{qc}_{m}", name=f"o{qc}_{m}")
                for m in range(NPF)] for qc in range(NQC)]

    # mask tiles: per phase, two half-chunks [128, 8, QC] (keys x kt x q)
    m_tiles = {}

    def load_mask_half(pi, h):
        # host pre-permuted mask: row qc*128+p holds [kt, q] contiguous
        qc = pi % NQC
        mt = mpool.tile([128, NKT // 2, QC], bf16, tag="mask", name="mask")
        nc.sync.dma_start(
            out=mt,
            in_=maskT[qc * 128:(qc + 1) * 128, h * 8:(h + 1) * 8, :],
        )
        m_tiles.setdefault(pi, {})[h] = mt

    def load_x(xname):
        tiles = []
        for j in range(NJIN):
            xt = xpool.tile([128, S], bf16, tag="x", name="x")
            nc.sync.dma_start(out=xt, in_=xdrams[xname][j * 128:(j + 1) * 128, :])
            tiles.append(xt)
        return tiles

    # ---- DMA emission order = queue order: xq, masks(0), xk, xv, wo, mask(1)
    nc.sync.dma_start(out=wq_sb, in_=ws["wq"][:, :, :])
    nc.sync.dma_start(out=bqc_sb, in_=bs["bqc"][:, :])
    nc.sync.dma_start(out=wk_sb, in_=ws["wk"][:, :, :])
    nc.sync.dma_start(out=bkc_sb, in_=bs["bkc"][:, :])
    nc.sync.dma_start(out=wv_sb, in_=ws["wv"][:, :, :])
    nc.sync.dma_start(out=bv_sb, in_=bs["bv"][:, :])
    xq_tiles = load_x("xqT")
    xk_tiles = load_x("xkT")
    xv_chunks = []
    for g in range(NH):
        xt = xvsub.tile([128, NJIN, 256], bf16, tag="xv", name="xv")
        nc.sync.dma_start(out=xt, in_=xvP[g * 128:(g + 1) * 128, :, :])
        xv_chunks.append(xt)
    load_mask_half(0, 0)
    load_mask_half(0, 1)
    nc.sync.dma_start(out=wo_sb, in_=ws["wo"][:, :, :])
    load_mask_half(1, 0)

    for t in range(NKT):
        nc.vector.memset(v_sb[t], 1.0)  # ones column for the PV rowsum trick

    # ---- Q/K projection pass for m=0 and m=1, j-streamed over arriving x
    def qk_pass01(x_tiles, w_sb, bcp, t_out):
        pas = [ps_a.tile([128, 2, QC], f32, tag="sc", name="sc") for _ in range(2)]
        pbs = [ps_b.tile([128, QC], f32, tag="pb", name="pb") for _ in range(NQC)]
        for j in range(NJIN):
            for h in range(2):
                for i in range(2):
                    nc.tensor.matmul(
                        pas[h][:, i, :],
                        lhsT=w_sb[:, j, 0:128],
                        rhs=x_tiles[j][:, (2 * h + i) * QC:(2 * h + i + 1) * QC],
                        start=(j == 0),
                        stop=(j == NJIN - 1),
                    )
            for n in range(NQC):
                nc.tensor.matmul(
                    pbs[n],
                    lhsT=w_sb[:, j, 128:256],
                    rhs=x_tiles[j][:, n * QC:(n + 1) * QC],
                    start=(j == 0),
                    stop=(j == NJIN - 1),
                )
        for h in range(2):
            nc.scalar.add(
                out=t_out[0][:, 2 * h * QC:(2 * h + 2) * QC],
                in_=pas[h].rearrange("p a b -> p (a b)"),
                add=bcp[:, 0:1])
        for n in range(NQC):
            nc.vector.tensor_scalar_add(
                out=t_out[1][:, n * QC:(n + 1) * QC], in0=pbs[n],
                scalar1=bcp[:, 1:2])

    qk_pass01(xq_tiles, wq_sb, bqc_sb, qt_sb)
    qk_pass01(xk_tiles, wk_sb, bkc_sb, kt_sb)

    # ---- V projection in 2-tile groups, streamed against the xvP chunks
    for g in range(NH):
        xt = xv_chunks[g]
        pss = [ps_b.tile([128, QC], f32, tag="pb", name="pb") for _ in range(2)]
        for j in range(NJIN):
            for i in range(2):
                nc.tensor.matmul(
                    pss[i],
                    lhsT=xt[:, j, i * 128:(i + 1) * 128],
                    rhs=wv_sb[:, j, :],
                    start=(j == 0),
                    stop=False,
                )
        for i in range(2):
            nc.tensor.matmul(pss[i], lhsT=ones_col, rhs=bv_sb,
                             start=False, stop=True)
        for i in range(2):
            vt = v_sb[2 * g + i]
            vview = bass.AP(tensor=vt.tensor, offset=vt.offset,
                            ap=[vt.ap[0], [DH + 1, NH], [1, DH]])
            nc.vector.tensor_scalar_mul(
                out=vview,
                in0=pss[i].rearrange("p (h d) -> p h d", h=NH),
                scalar1=1.0)

    # ---- in-loop fillers: Q/K m2,m3 from re-DMA'd x; then Y as it queues
    fillers = deque()
    fstate = {}

    def redma(xname):
        def run():
            fstate[xname] = load_x(xname)
        return run

    def qk_group(xname, w_sb, bcp, t_out, m, n):
        def run():
            x_tiles = fstate[xname]
            ps = ps_b.tile([128, QC], f32, tag="pb", name="pb")
            for j in range(NJIN):
                nc.tensor.matmul(
                    ps,
                    lhsT=w_sb[:, j, m * 128:(m + 1) * 128],
                    rhs=x_tiles[j][:, n * QC:(n + 1) * QC],
                    start=(j == 0),
                    stop=(j == NJIN - 1),
                )
            nc.vector.tensor_scalar_add(
                out=t_out[m][:, n * QC:(n + 1) * QC], in0=ps,
                scalar1=bcp[:, m:m + 1])
        return run

    fillers.append(redma("xqT"))
    for m in (2, 3):
        for n in range(NQC):
            fillers.append(qk_group("xqT", wq_sb, bqc_sb, qt_sb, m, n))
    fillers.append(redma("xkT"))
    for m in (2, 3):
        for n in range(NQC):
            fillers.append(qk_group("xkT", wk_sb, bkc_sb, kt_sb, m, n))
    y_fill = deque()

    # ---- main software-pipelined stream (phases tp-major: pi = tp*NQC + qc)
    pv_pss = {}       # pi -> [2 psum accumulators]
    p2s = {}          # step -> p2 tile
    norm_b_queue = []
    ysbs = {}

    def y_half(qc, qt, nch):
        def run():
            if nch == 0:
                ysbs[(qc, qt)] = ypool.tile([128, 2, QC], f32, tag="y", name="y")
            ysb = ysbs[(qc, qt)]
            ps = ps_b.tile([128, QC], f32, tag="pb", name="pb")
            for j in range(NPF):
                nc.tensor.matmul(
                    ps,
                    lhsT=out_sbs[qc][j][:, qt * 128:(qt + 1) * 128],
                    rhs=wo_sb[:, j, nch * QC:(nch + 1) * QC],
                    start=(j == 0),
                    stop=(j == NPF - 1),
                )
            nc.vector.tensor_scalar_mul(out=ysb[:, nch, :], in0=ps, scalar1=1.0)
            if nch == 1:
                del ysbs[(qc, qt)]
                r0 = qc * QC + qt * 128
                nc.sync.dma_start(out=y[r0:r0 + 128, :], in_=ysb)
        return run

    def lag(x):
        return 8 if x < 16 else max(4, 24 - x)

    def emit_sk(s):
        pi, kt = divmod(s, NKT)
        tp, qc = pi // NQC, pi % NQC
        if kt == 4 and pi + 1 < NPH and 0 not in m_tiles.get(pi + 1, {}):
            load_mask_half(pi + 1, 0)
        if kt == 10 and pi + 1 < NPH:
            load_mask_half(pi + 1, 1)
        qsl = slice(qc * QC, (qc + 1) * QC)
        sc = ps_a.tile([128, 2, QC], f32, tag="sc", name="sc")
        for sub in range(2):
            rsl = slice(sub * 64, (sub + 1) * 64)
            nc.tensor.matmul(
                sc[:, sub, :],
                lhsT=kt_sb[tp][rsl, kt * 128:(kt + 1) * 128],
                rhs=qt_sb[tp][rsl, qsl],
                start=True,
                stop=True,
            )
        p2 = ppool.tile([128, 2, QC], bf16, tag="p", name="p")
        nc.scalar.activation(out=p2, in_=sc, func=EXPF, scale=0.125)
        # one masked multiply for both heads: mask broadcast over the head dim
        mh = m_tiles[pi][kt // 8][:, kt % 8, :]
        mask_bc = bass.AP(tensor=mh.tensor, offset=mh.offset,
                          ap=[mh.ap[0], [0, 2], mh.ap[1]])
        nc.vector.tensor_tensor(out=p2, in0=p2, in1=mask_bc,
                                op=mybir.AluOpType.mult)
        p2s[s] = p2

    def emit_pv(s):
        pi, kt = divmod(s, NKT)
        tp, qc = pi // NQC, pi % NQC
        if kt == 0:
            pv_pss[pi] = [ps_b.tile([128, QC], f32, tag="pb", name="pb")
                          for _ in range(2)]
        p2 = p2s.pop(s)
        for sub in range(2):
            h0 = (2 * tp + sub) * (DH + 1)
            nc.tensor.matmul(
                pv_pss[pi][sub][0:128, :],
                lhsT=v_sb[kt][:, h0:h0 + 128],
                rhs=p2[:, sub, :],
                start=(kt == 0),
                stop=(kt == NKT - 1),
            )
        if kt == NKT - 1:
            emit_norm_a(pi, s)

    def norm_rec(pi, sub, recbs):
        # rowsums live at psum partition 64 of each PV accumulator
        rsum = norm_s.tile([1, QC], f32, tag=f"rsum{sub}", name="rsum")
        nc.vector.tensor_copy(out=rsum, in_=pv_pss[pi][sub][DH:DH + 1, :])
        rec = norm_s.tile([1, QC], f32, tag=f"rec{sub}", name="rec")
        scr = norm_s.tile([1, QC], f32, tag="scr", name="scr")
        nc.vector.reciprocal_approx_accurate(out=rec, in_=rsum, scratch=scr)
        recb = norm_r.tile([64, QC], f32, tag=f"recb{sub}", name="recb")
        rec_bc = bass.AP(tensor=rec.tensor, offset=rec.offset,
                         ap=[rec.ap[0], [0, 64], rec.ap[1]])
        # idle GpSimd SWDGE queue: skips the busy sync-DMA queue backlog
        nc.gpsimd.dma_start(out=recb, in_=rec_bc)
        recbs.append(recb)

    def norm_mult(pi, sub, recbs):
        tp, qc = pi // NQC, pi % NQC
        rsl = slice(sub * 64, (sub + 1) * 64)
        nc.vector.tensor_tensor(
            out=out_sbs[qc][tp][rsl, :],
            in0=pv_pss[pi][sub][0:DH, :],
            in1=recbs[sub],
            op=mybir.AluOpType.mult,
        )
        if sub == 1:
            pv_pss.pop(pi)
            tpq, qcq = pi // NQC, pi % NQC
            if tpq == NPF - 1:
                for qt in range(NQC):
                    for nch in range(2):
                        y_fill.append(y_half(qcq, qt, nch))

    def emit_norm_a(pi, s):
        # spread the norm work one op-group per step to avoid a DVE burst
        recbs = []
        norm_b_queue.append((s + 1, lambda: norm_rec(pi, 0, recbs)))
        norm_b_queue.append((s + 2, lambda: norm_rec(pi, 1, recbs)))
        norm_b_queue.append((s + 8, lambda: norm_mult(pi, 0, recbs)))
        norm_b_queue.append((s + 9, lambda: norm_mult(pi, 1, recbs)))

    def emit_norm_b():
        _, fn = norm_b_queue.pop(0)
        fn()

    pv_next = 0
    for s in range(NSTEP + 96):
        if s < NSTEP:
            emit_sk(s)
        while norm_b_queue and norm_b_queue[0][0] <= s:
            emit_norm_b()
        while pv_next < NSTEP and pv_next <= s - lag(pv_next):
            emit_pv(pv_next)
            pv_next += 1
        while norm_b_queue and norm_b_queue[0][0] <= s:
            emit_norm_b()
        # fillers: keep clear of the PV-accumulator handover window (kt 4-8)
        kt = s % NKT
        if not (4 <= kt <= 8) or s >= NSTEP:
            if fillers and (s >= 28 and s % 3 == 0 or s >= NSTEP):
                fillers.popleft()()
            elif y_fill:
                y_fill.popleft()()
                if s >= NSTEP and y_fill:
                    y_fill.popleft()()
        if s >= NSTEP and pv_next >= NSTEP and not norm_b_queue and \
                not fillers and not y_fill:
            break
    while norm_b_queue:
        emit_norm_b()
    while y_fill:
        y_fill.popleft()()


_NC_CACHE = None


def _build_nc():
    global _NC_CACHE
    if _NC_CACHE is None:
        nc = bacc.Bacc("TRN2", target_bir_lowering=False, name="mhsa")
        xdrams = {
            n: nc.declare_dram_parameter(n, [HIN, S], bf16, isOutput=False)
            for n in ("xqT", "xkT")
        }
        maskT = nc.declare_dram_parameter(
            "maskT", [NQC * 128, NKT, QC], bf16, isOutput=False)
        xvP = nc.declare_dram_parameter(
            "xvP", [NH * 128, NJIN, 256], bf16, isOutput=False)
        ws = {
            "wq": nc.declare_dram_parameter("wq", [128, NJIN, F], bf16, isOutput=False),
            "wk": nc.declare_dram_parameter("wk", [128, NJIN, F], bf16, isOutput=False),
            "wv": nc.declare_dram_parameter("wv", [128, NJIN, F], bf16, isOutput=False),
            "wo": nc.declare_dram_parameter("wo", [128, NPF, HOUT], bf16, isOutput=False),
        }
        bs = {
            "bqc": nc.declare_dram_parameter("bqc", [128, NPF], f32, isOutput=False),
            "bkc": nc.declare_dram_parameter("bkc", [128, NPF], f32, isOutput=False),
            "bv": nc.declare_dram_parameter("bv", [1, F], bf16, isOutput=False),
        }
        y = nc.declare_dram_parameter("y", [S, HOUT], f32, isOutput=True)
        with tile.TileContext(nc) as tc:
            with ExitStack() as ctx:
                _attention_body(ctx, tc, (xdrams, maskT, xvP, ws, bs, y))
        nc.compile()
        _NC_CACHE = nc
    return _NC_CACHE


LAST_RESULTS = None


def kernel(queries, keys, values, attention_mask,
           W_q, b_q, W_k, b_k, W_v, b_v, W_o, b_o):
    global LAST_RESULTS
    nc = _build_nc()

    B = queries.shape[0]
    n_cores = 2 * B

    def prep_w(W, g):
        Wg = np.asarray(W[:, g * F:(g + 1) * F], np.float32).astype(BF16)
        return np.ascontiguousarray(Wg.reshape(NJIN, 128, F).transpose(1, 0, 2))

    def prep_wo(W, g):
        Wg = np.asarray(W[g * F:(g + 1) * F, :], np.float32).astype(BF16)
        return np.ascontiguousarray(Wg.reshape(NPF, 128, HOUT).transpose(1, 0, 2))

    def prep_bcol(b, g):
        bg = np.asarray(b[g * F:(g + 1) * F], np.float32)
        return np.ascontiguousarray(bg.reshape(NPF, 128).T)

    in_maps = []
    for b in range(B):
        xqT_ = np.ascontiguousarray(np.asarray(queries[b], np.float32).astype(BF16).T)
        xkT_ = np.ascontiguousarray(np.asarray(keys[b], np.float32).astype(BF16).T)
        # xvP[g*128+p, j, tok] = X_v^T[j*128+p, g*256+tok] (contiguous V chunks)
        xvP_ = np.ascontiguousarray(
            np.asarray(values[b], np.float32).astype(BF16).T
            .reshape(NJIN, 128, NH, 256).transpose(2, 1, 0, 3)
            .reshape(NH * 128, NJIN, 256))
        # permute mask so each (qc, key-tile-half) chunk is DMA-contiguous:
        # maskT_[qc*128+p, t, ql] = mask[qc*512+ql, t*128+p]
        maskT_ = np.ascontiguousarray(
            np.asarray(attention_mask[b]).astype(np.float32).astype(BF16)
            .reshape(NQC, QC, NKT, 128).transpose(0, 3, 2, 1)
            .reshape(NQC * 128, NKT, QC))
        for g in range(2):
            in_maps.append({
                "xqT": xqT_, "xkT": xkT_, "xvP": xvP_, "maskT": maskT_,
                "wq": prep_w(W_q, g), "wk": prep_w(W_k, g), "wv": prep_w(W_v, g),
                "wo": prep_wo(W_o, g),
                "bqc": prep_bcol(b_q, g),
                "bkc": prep_bcol(b_k, g),
                "bv": np.asarray(b_v[g * F:(g + 1) * F], np.float32).astype(BF16).reshape(1, F),
            })

    res = run_bass_kernel_spmd(
        nc, in_maps, list(range(n_cores)),
        trace=bool(os.environ.get("MHSA_TRACE")),
    )
    LAST_RESULTS = res

    out = np.empty((B, S, HOUT), np.float32)
    bo = np.asarray(b_o, np.float32)
    for b in range(B):
        out[b] = res.results[2 * b]["y"] + res.results[2 * b + 1]["y"] + bo
    return out


# revision 76
# speedup vs baseline: 1.0125x; 1.0030x over previous
"""Multi-head self-attention Trainium2 kernel (8 NeuronCores, SPMD).

Problem: B=4, S=2048, H=1024, 16 heads (dh=64), fp32 I/O.
Sharding: core c = b*2 + g handles batch b and head-group g (8 heads).
Each core computes a partial output Y_g = softmax(QK^T/sqrt(d), mask) V W_o[g]
for its 8 heads; the host sums the two partials per batch and adds b_o.

v2 design notes (from NTFF profile analysis of v1):
- ACT (exp over all 33.5M score elems/core) is a hard ~300us floor at
  1 elem/lane/cycle @1.2GHz; the goal is to keep every other engine hidden
  underneath back-to-back exps.
- The two K=64 score matmuls of a head pair auto-row-tile (base partitions
  0/64) and run concurrently in the PE array.
- Phases are tp-major (pi = tp*4 + qc) so only head-pair 0's Q/K
  projections gate loop start; m2/m3 are re-projected in-loop from
  re-DMA'd x tiles (whole-row [128,S] DMAs only - chunked column DMAs
  have 1KB segments and run ~4x slower).
- Tile's scheduler reorders by priority (emission order) + cost model;
  buffers are sized so it can pack V projection and Q/K m1 fillers under
  the ACT-bound steady state.
- PSUM: ps_a = 2 x [128,2,512] score tiles (4 banks, double buffer);
  ps_b = 4 x [128,512] (PV accumulators + projection/Y transients).
"""

import os
import sys
from collections import deque
from contextlib import ExitStack

sys.path.insert(0, "/opt/trn_rl_repo")

import numpy as np
import ml_dtypes

import concourse.bass as bass
import concourse.tile as tile
from concourse import bacc
from concourse import mybir
from concourse.bass_utils import run_bass_kernel_spmd

BF16 = ml_dtypes.bfloat16

# Geometry (hardcoded for this problem)
S = 2048          # sequence length
HIN = 1024        # model hidden
F = 512           # per-core features = 8 heads * 64
NH = 8            # heads per core
DH = 64           # head dim
HOUT = 1024       # output hidden
QC = 512          # q chunk
NQC = S // QC     # 4
NKT = 16          # key tiles of 128
NJIN = HIN // 128  # 8 contraction tiles for projections
NPF = F // 128     # 4 feature ptiles (2 heads each)
NPH = NPF * NQC    # 16 phases
NSTEP = NPH * NKT  # 256

f32 = mybir.dt.float32
bf16 = mybir.dt.bfloat16
EXPF = mybir.ActivationFunctionType.Exp


def _attention_body(ctx, tc, io):
    nc = tc.nc
    xdrams, maskT, xvP, ws, bs, y = io

    consts = ctx.enter_context(tc.tile_pool(name="consts", bufs=1))
    wpool = ctx.enter_context(tc.tile_pool(name="wpool", bufs=1))
    xpool = ctx.enter_context(tc.tile_pool(name="xpool", bufs=8))
    xvsub = ctx.enter_context(tc.tile_pool(name="xvsub", bufs=2))
    qkvp = ctx.enter_context(tc.tile_pool(name="qkvp", bufs=1))
    mpool = ctx.enter_context(tc.tile_pool(name="mpool", bufs=3))
    ppool = ctx.enter_context(tc.tile_pool(name="ppool", bufs=11))
    outp = ctx.enter_context(tc.tile_pool(name="outp", bufs=1))
    ypool = ctx.enter_context(tc.tile_pool(name="ypool", bufs=2))
    norm_s = ctx.enter_context(tc.tile_pool(name="norm_s", bufs=1))
    norm_r = ctx.enter_context(tc.tile_pool(name="norm_r", bufs=1))
    ps_a = ctx.enter_context(tc.tile_pool(name="ps_a", bufs=2, space="PSUM"))
    ps_b = ctx.enter_context(tc.tile_pool(name="ps_b", bufs=4, space="PSUM"))

    # ---- constants + ACT exp-table warmup (table load ~2.7us hides in DMA)
    warm_in = consts.tile([1, 32], f32, tag="warm_in", name="warm_in")
    warm_out = consts.tile([1, 32], bf16, tag="warm_out", name="warm_out")
    nc.vector.memset(warm_in, 0.0)
    nc.scalar.activation(out=warm_out, in_=warm_in, func=EXPF, scale=0.125)

    ones_col = consts.tile([1, 128], bf16, tag="ones_col", name="ones_col")
    nc.vector.memset(ones_col, 1.0)

    wq_sb = wpool.tile([128, NJIN, F], bf16, tag="wq", name="wq")
    wk_sb = wpool.tile([128, NJIN, F], bf16, tag="wk", name="wk")
    wv_sb = wpool.tile([128, NJIN, F], bf16, tag="wv", name="wv")
    wo_sb = wpool.tile([128, NPF, HOUT], bf16, tag="wo", name="wo")
    bqc_sb = consts.tile([128, NPF], f32, tag="bqc", name="bqc")
    bkc_sb = consts.tile([128, NPF], f32, tag="bkc", name="bkc")
    bv_sb = consts.tile([1, F], bf16, tag="bv", name="bv")

    # ---- persistent SBUF tensors
    qt_sb = [qkvp.tile([128, S], bf16, tag=f"qt{m}", name=f"qt{m}") for m in range(NPF)]
    kt_sb = [qkvp.tile([128, S], bf16, tag=f"kt{m}", name=f"kt{m}") for m in range(NPF)]
    # flat V tiles: head h at columns [h*65, h*65+64]; padded to 583 so a
    # 128-column stationary window (FWL-eligible) can start at any head
    v_sb = [qkvp.tile([128, NH * (DH + 1) + 63], bf16, tag=f"v{t}", name=f"v{t}")
            for t in range(NKT)]
    out_sbs = [[outp.tile([128, QC], bf16, tag=f"o{qc}_{m}", name=f"o{qc}_{m}")
                for m in range(NPF)] for qc in range(NQC)]

    # mask tiles: per phase, two half-chunks [128, 8, QC] (keys x kt x q)
    m_tiles = {}

    def load_mask_half(pi, h):
        # host pre-permuted mask: row qc*128+p holds [kt, q] contiguous
        qc = pi % NQC
        mt = mpool.tile([128, NKT // 2, QC], bf16, tag="mask", name="mask")
        nc.sync.dma_start(
            out=mt,
            in_=maskT[qc * 128:(qc + 1) * 128, h * 8:(h + 1) * 8, :],
        )
        m_tiles.setdefault(pi, {})[h] = mt

    def load_x(xname):
        tiles = []
        for j in range(NJIN):
            xt = xpool.tile([128, S], bf16, tag="x", name="x")
            nc.sync.dma_start(out=xt, in_=xdrams[xname][j * 128:(j + 1) * 128, :])
            tiles.append(xt)
        return tiles

    # ---- DMA emission order = queue order: xq, masks(0), xk, xv, wo, mask(1)
    nc.sync.dma_start(out=wq_sb, in_=ws["wq"][:, :, :])
    nc.sync.dma_start(out=bqc_sb, in_=bs["bqc"][:, :])
    nc.sync.dma_start(out=wk_sb, in_=ws["wk"][:, :, :])
    nc.sync.dma_start(out=bkc_sb, in_=bs["bkc"][:, :])
    nc.sync.dma_start(out=wv_sb, in_=ws["wv"][:, :, :])
    nc.sync.dma_start(out=bv_sb, in_=bs["bv"][:, :])
    xq_tiles = load_x("xqT")
    xk_tiles = load_x("xkT")
    xv_chunks = []
    for g in range(NH):
        xt = xvsub.tile([128, NJIN, 256], bf16, tag="xv", name="xv")
        nc.sync.dma_start(out=xt, in_=xvP[g * 128:(g + 1) * 128, :, :])
        xv_chunks.append(xt)
    load_mask_half(0, 0)
    load_mask_half(0, 1)
    nc.sync.dma_start(out=wo_sb, in_=ws["wo"][:, :, :])
    load_mask_half(1, 0)

    for t in range(NKT):
        nc.vector.memset(v_sb[t], 1.0)  # ones column for the PV rowsum trick

    # ---- Q/K projection pass for m=0 and m=1, j-streamed over arriving x
    def qk_pass01(x_tiles, w_sb, bcp, t_out):
        pas = [ps_a.tile([128, 2, QC], f32, tag="sc", name="sc") for _ in range(2)]
        pbs = [ps_b.tile([128, QC], f32, tag="pb", name="pb") for _ in range(NQC)]
        for j in range(NJIN):
            for h in range(2):
                for i in range(2):
                    nc.tensor.matmul(
                        pas[h][:, i, :],
                        lhsT=w_sb[:, j, 0:128],
                        rhs=x_tiles[j][:, (2 * h + i) * QC:(2 * h + i + 1) * QC],
                        start=(j == 0),
                        stop=(j == NJIN - 1),
                    )
            for n in range(NQC):
                nc.tensor.matmul(
                    pbs[n],
                    lhsT=w_sb[:, j, 128:256],
                    rhs=x_tiles[j][:, n * QC:(n + 1) * QC],
                    start=(j == 0),
                    stop=(j == NJIN - 1),
                )
        for h in range(2):
            nc.scalar.add(
                out=t_out[0][:, 2 * h * QC:(2 * h + 2) * QC],
                in_=pas[h].rearrange("p a b -> p (a b)"),
                add=bcp[:, 0:1])
        for n in range(NQC):
            nc.vector.tensor_scalar_add(
                out=t_out[1][:, n * QC:(n + 1) * QC], in0=pbs[n],
                scalar1=bcp[:, 1:2])

    qk_pass01(xq_tiles, wq_sb, bqc_sb, qt_sb)
    qk_pass01(xk_tiles, wk_sb, bkc_sb, kt_sb)

    # ---- V projection in 2-tile groups, streamed against the xvP chunks
    for g in range(NH):
        xt = xv_chunks[g]
        pss = [ps_b.tile([128, QC], f32, tag="pb", name="pb") for _ in range(2)]
        for j in range(NJIN):
            for i in range(2):
                nc.tensor.matmul(
                    pss[i],
                    lhsT=xt[:, j, i * 128:(i + 1) * 128],
                    rhs=wv_sb[:, j, :],
                    start=(j == 0),
                    stop=False,
                )
        for i in range(2):
            nc.tensor.matmul(pss[i], lhsT=ones_col, rhs=bv_sb,
                             start=False, stop=True)
        for i in range(2):
            vt = v_sb[2 * g + i]
            vview = bass.AP(tensor=vt.tensor, offset=vt.offset,
                            ap=[vt.ap[0], [DH + 1, NH], [1, DH]])
            nc.vector.tensor_scalar_mul(
                out=vview,
                in0=pss[i].rearrange("p (h d) -> p h d", h=NH),
                scalar1=1.0)

    # ---- in-loop fillers: Q/K m2,m3 from re-DMA'd x; then Y as it queues
    fillers = deque()
    fstate = {}

    def redma(xname):
        def run():
            fstate[xname] = load_x(xname)
        return run

    def qk_group(xname, w_sb, bcp, t_out, m, n):
        def run():
            x_tiles = fstate[xname]
            ps = ps_b.tile([128, QC], f32, tag="pb", name="pb")
            for j in range(NJIN):
                nc.tensor.matmul(
                    ps,
                    lhsT=w_sb[:, j, m * 128:(m + 1) * 128],
                    rhs=x_tiles[j][:, n * QC:(n + 1) * QC],
                    start=(j == 0),
                    stop=(j == NJIN - 1),
                )
            nc.vector.tensor_scalar_add(
                out=t_out[m][:, n * QC:(n + 1) * QC], in0=ps,
                scalar1=bcp[:, m:m + 1])
        return run

    fillers.append(redma("xqT"))
    for m in (2, 3):
        for n in range(NQC):
            fillers.append(qk_group("xqT", wq_sb, bqc_sb, qt_sb, m, n))
    fillers.append(redma("xkT"))
    for m in (2, 3):
        for n in range(NQC):
            fillers.append(qk_group("xkT", wk_sb, bkc_sb, kt_sb, m, n))
    y_fill = deque()

    # ---- main software-pipelined stream (phases tp-major: pi = tp*NQC + qc)
    pv_pss = {}       # pi -> [2 psum accumulators]
    p2s = {}          # step -> p2 tile
    norm_b_queue = []
    ysbs = {}

    def y_half(qc, qt, nch):
        def run():
            if nch == 0:
                ysbs[(qc, qt)] = ypool.tile([128, 2, QC], f32, tag="y", name="y")
            ysb = ysbs[(qc, qt)]
            ps = ps_b.tile([128, QC], f32, tag="pb", name="pb")
            for j in range(NPF):
                nc.tensor.matmul(
                    ps,
                    lhsT=out_sbs[qc][j][:, qt * 128:(qt + 1) * 128],
                    rhs=wo_sb[:, j, nch * QC:(nch + 1) * QC],
                    start=(j == 0),
                    stop=(j == NPF - 1),
                )
            nc.vector.tensor_scalar_mul(out=ysb[:, nch, :], in0=ps, scalar1=1.0)
            if nch == 1:
                del ysbs[(qc, qt)]
                r0 = qc * QC + qt * 128
                nc.sync.dma_start(out=y[r0:r0 + 128, :], in_=ysb)
        return run

    def lag(x):
        return 8 if x < 16 else max(4, 24 - x)

    def emit_sk(s):
        pi, kt = divmod(s, NKT)
        tp, qc = pi // NQC, pi % NQC
        if kt == 4 and pi + 1 < NPH and 0 not in m_tiles.get(pi + 1, {}):
            load_mask_half(pi + 1, 0)
        if kt == 10 and pi + 1 < NPH:
            load_mask_half(pi + 1, 1)
        qsl = slice(qc * QC, (qc + 1) * QC)
        sc = ps_a.tile([128, 2, QC], f32, tag="sc", name="sc")
        for sub in range(2):
            rsl = slice(sub * 64, (sub + 1) * 64)
            nc.tensor.matmul(
                sc[:, sub, :],
                lhsT=kt_sb[tp][rsl, kt * 128:(kt + 1) * 128],
                rhs=qt_sb[tp][rsl, qsl],
                start=True,
                stop=True,
            )
        p2 = ppool.tile([128, 2, QC], bf16, tag="p", name="p")
        nc.scalar.activation(out=p2, in_=sc, func=EXPF, scale=0.125)
        # one masked multiply for both heads: mask broadcast over the head dim
        mh = m_tiles[pi][kt // 8][:, kt % 8, :]
        mask_bc = bass.AP(tensor=mh.tensor, offset=mh.offset,
                          ap=[mh.ap[0], [0, 2], mh.ap[1]])
        nc.vector.tensor_tensor(out=p2, in0=p2, in1=mask_bc,
                                op=mybir.AluOpType.mult)
        p2s[s] = p2

    def emit_pv(s):
        pi, kt = divmod(s, NKT)
        tp, qc = pi // NQC, pi % NQC
        if kt == 0:
            pv_pss[pi] = [ps_b.tile([128, QC], f32, tag="pb", name="pb")
                          for _ in range(2)]
        p2 = p2s.pop(s)
        for sub in range(2):
            h0 = (2 * tp + sub) * (DH + 1)
            nc.tensor.matmul(
                pv_pss[pi][sub][0:128, :],
                lhsT=v_sb[kt][:, h0:h0 + 128],
                rhs=p2[:, sub, :],
                start=(kt == 0),
                stop=(kt == NKT - 1),
            )
        if kt == NKT - 1:
            emit_norm_a(pi, s)

    def norm_rec(pi, sub, recbs):
        # rowsums live at psum partition 64 of each PV accumulator
        rsum = norm_s.tile([1, QC], f32, tag="rsum", name="rsum")
        nc.vector.tensor_copy(out=rsum, in_=pv_pss[pi][sub][DH:DH + 1, :])
        rec = norm_s.tile([1, QC], f32, tag=f"rec{sub}", name="rec")
        scr = norm_s.tile([1, QC], f32, tag="scr", name="scr")
        nc.vector.reciprocal_approx_accurate(out=rec, in_=rsum, scratch=scr)
        recb = norm_r.tile([64, QC], f32, tag=f"recb{sub}", name="recb")
        rec_bc = bass.AP(tensor=rec.tensor, offset=rec.offset,
                         ap=[rec.ap[0], [0, 64], rec.ap[1]])
        # idle GpSimd SWDGE queue: skips the busy sync-DMA queue backlog
        nc.gpsimd.dma_start(out=recb, in_=rec_bc)
        recbs.append(recb)

    def norm_mult(pi, sub, recbs):
        tp, qc = pi // NQC, pi % NQC
        rsl = slice(sub * 64, (sub + 1) * 64)
        nc.vector.tensor_tensor(
            out=out_sbs[qc][tp][rsl, :],
            in0=pv_pss[pi][sub][0:DH, :],
            in1=recbs[sub],
            op=mybir.AluOpType.mult,
        )
        if sub == 1:
            pv_pss.pop(pi)
            tpq, qcq = pi // NQC, pi % NQC
            if tpq == NPF - 1:
                for qt in range(NQC):
                    for nch in range(2):
                        y_fill.append(y_half(qcq, qt, nch))

    def emit_norm_a(pi, s):
        # spread the norm work one op-group per step to avoid a DVE burst
        recbs = []
        norm_b_queue.append((s + 1, lambda: norm_rec(pi, 0, recbs)))
        norm_b_queue.append((s + 2, lambda: norm_rec(pi, 1, recbs)))
        norm_b_queue.append((s + 8, lambda: norm_mult(pi, 0, recbs)))
        norm_b_queue.append((s + 9, lambda: norm_mult(pi, 1, recbs)))

    def emit_norm_b():
        _, fn = norm_b_queue.pop(0)
        fn()

    pv_next = 0
    for s in range(NSTEP + 96):
        if s < NSTEP:
            emit_sk(s)
        while norm_b_queue and norm_b_queue[0][0] <= s:
            emit_norm_b()
        while pv_next < NSTEP and pv_next <= s - lag(pv_next):
            emit_pv(pv_next)
            pv_next += 1
        while norm_b_queue and norm_b_queue[0][0] <= s:
            emit_norm_b()
        # fillers: keep clear of the PV-accumulator handover window (kt 4-8)
        kt = s % NKT
        if not (4 <= kt <= 8) or s >= NSTEP:
            if fillers and (s >= 28 and s % 3 == 0 or s >= NSTEP):
                fillers.popleft()()
            elif y_fill:
                y_fill.popleft()()
                if s >= NSTEP and y_fill:
                    y_fill.popleft()()
        if s >= NSTEP and pv_next >= NSTEP and not norm_b_queue and \
                not fillers and not y_fill:
            break
    while norm_b_queue:
        emit_norm_b()
    while y_fill:
        y_fill.popleft()()


_NC_CACHE = None


def _build_nc():
    global _NC_CACHE
    if _NC_CACHE is None:
        nc = bacc.Bacc("TRN2", target_bir_lowering=False, name="mhsa")
        xdrams = {
            n: nc.declare_dram_parameter(n, [HIN, S], bf16, isOutput=False)
            for n in ("xqT", "xkT")
        }
        maskT = nc.declare_dram_parameter(
            "maskT", [NQC * 128, NKT, QC], bf16, isOutput=False)
        xvP = nc.declare_dram_parameter(
            "xvP", [NH * 128, NJIN, 256], bf16, isOutput=False)
        ws = {
            "wq": nc.declare_dram_parameter("wq", [128, NJIN, F], bf16, isOutput=False),
            "wk": nc.declare_dram_parameter("wk", [128, NJIN, F], bf16, isOutput=False),
            "wv": nc.declare_dram_parameter("wv", [128, NJIN, F], bf16, isOutput=False),
            "wo": nc.declare_dram_parameter("wo", [128, NPF, HOUT], bf16, isOutput=False),
        }
        bs = {
            "bqc": nc.declare_dram_parameter("bqc", [128, NPF], f32, isOutput=False),
            "bkc": nc.declare_dram_parameter("bkc", [128, NPF], f32, isOutput=False),
            "bv": nc.declare_dram_parameter("bv", [1, F], bf16, isOutput=False),
        }
        y = nc.declare_dram_parameter("y", [S, HOUT], f32, isOutput=True)
        with tile.TileContext(nc) as tc:
            with ExitStack() as ctx:
                _attention_body(ctx, tc, (xdrams, maskT, xvP, ws, bs, y))
        nc.compile()
        _NC_CACHE = nc
    return _NC_CACHE


LAST_RESULTS = None


def kernel(queries, keys, values, attention_mask,
           W_q, b_q, W_k, b_k, W_v, b_v, W_o, b_o):
    global LAST_RESULTS
    nc = _build_nc()

    B = queries.shape[0]
    n_cores = 2 * B

    def prep_w(W, g):
        Wg = np.asarray(W[:, g * F:(g + 1) * F], np.float32).astype(BF16)
        return np.ascontiguousarray(Wg.reshape(NJIN, 128, F).transpose(1, 0, 2))

    def prep_wo(W, g):
        Wg = np.asarray(W[g * F:(g + 1) * F, :], np.float32).astype(BF16)
        return np.ascontiguousarray(Wg.reshape(NPF, 128, HOUT).transpose(1, 0, 2))

    def prep_bcol(b, g):
        bg = np.asarray(b[g * F:(g + 1) * F], np.float32)
        return np.ascontiguousarray(bg.reshape(NPF, 128).T)

    in_maps = []
    for b in range(B):
        xqT_ = np.ascontiguousarray(np.asarray(queries[b], np.float32).astype(BF16).T)
        xkT_ = np.ascontiguousarray(np.asarray(keys[b], np.float32).astype(BF16).T)
        # xvP[g*128+p, j, tok] = X_v^T[j*128+p, g*256+tok] (contiguous V chunks)
        xvP_ = np.ascontiguousarray(
            np.asarray(values[b], np.float32).astype(BF16).T
            .reshape(NJIN, 128, NH, 256).transpose(2, 1, 0, 3)
            .reshape(NH * 128, NJIN, 256))
        # permute mask so each (qc, key-tile-half) chunk is DMA-contiguous:
        # maskT_[qc*128+p, t, ql] = mask[qc*512+ql, t*128+p]
        maskT_ = np.ascontiguousarray(
            np.asarray(attention_mask[b]).astype(np.float32).astype(BF16)
            .reshape(NQC, QC, NKT, 128).transpose(0, 3, 2, 1)
            .reshape(NQC * 128, NKT, QC))
        for g in range(2):
            in_maps.append({
                "xqT": xqT_, "xkT": xkT_, "xvP": xvP_, "maskT": maskT_,
                "wq": prep_w(W_q, g), "wk": prep_w(W_k, g), "wv": prep_w(W_v, g),
                "wo": prep_wo(W_o, g),
                "bqc": prep_bcol(b_q, g),
                "bkc": prep_bcol(b_k, g),
                "bv": np.asarray(b_v[g * F:(g + 1) * F], np.float32).astype(BF16).reshape(1, F),
            })

    res = run_bass_kernel_spmd(
        nc, in_maps, list(range(n_cores)),
        trace=bool(os.environ.get("MHSA_TRACE")),
    )
    LAST_RESULTS = res

    out = np.empty((B, S, HOUT), np.float32)
    bo = np.asarray(b_o, np.float32)
    for b in range(B):
        out[b] = res.results[2 * b]["y"] + res.results[2 * b + 1]["y"] + bo
    return out
